# revision 10
# baseline (speedup 1.0000x reference)
"""Trainium2 Bass kernel for the rank-weighted hard-negative hinge loss.

Math (reference):
    scores = im @ s.T                         # [N, N]
    diag   = diagonal(scores)
    rank1[i] = #{j : scores[i,j] < diag[i]}   (row rank of diag)
    rank2[j] = #{i : scores[i,j] < diag[j]}   (col rank of diag)
    cost_s  = 1/(rank1+1) * max_j!=i relu(M + scores[i,j] - diag[i])
    cost_im = 1/(rank2+1) * max_i!=j relu(M + scores[i,j] - diag[j])
    loss = sum(cost_s) + sum(cost_im)

v4 strategy:
  - Scores from bf16-rounded inputs, fp32 PSUM accumulation (bf16 matmuls
    are 4x fp32 on the PE); diag from the same bf16 inputs keeps every
    comparison consistent (sim: rel err ~1.5e-3).
  - ACT owns the PSUM reads: a fp16 Copy conversion per tile, plus
    Sign(+accum) row-rank counts for superchunks 3..7.
  - DVE consumes the fp16 copy with 2x tensor_tensor only: ind2 = (S < d_j)
    is_lt, colmax and rowmax as in-place max accumulators (first touch is a
    4x tensor_copy, which also replaces the big memsets). Row counts for
    superchunks 0..2 are raw 4x tensor_scalar indicators DMA'd out and
    summed on the host (DMA engines are otherwise idle).
  - cnt2 = partition sums of ind2 on the PE (ones-matmul, accumulated over
    row tiles per superchunk). Cnt matmuls are emitted LAG iterations late
    so they never stall the PE on the DVE pipeline; a warm-up burst plus
    deeper PSUM buffering (3 score tiles) keeps the PE HAM at full clock.
  - Outputs are DMA'd in chunks as soon as their accumulation closes, so
    almost nothing is left for the tail.
  - The diagonal is masked by adding -1e30 into PSUM; it deterministically
    counts as "below diag" in both rank counts, yielding rank+1 exactly.

Sharding: core r owns rows [r*1024, (r+1)*1024). Each core receives s.T with
columns rotated left by r*1024 so the diagonal block sits at local column
offset = local row index on every core. Column stats are un-rotated on the
host, which also does the final reduction across cores.
"""

import numpy as np
import ml_dtypes

N = 8192
D = 256
NCORES = 8
RL = N // NCORES  # rows per core
MARGIN = 0.2
NEG = np.float32(-1.0e30)

SC_W = 1024            # column superchunk width
NSC = N // SC_W        # 8 superchunks
NT = RL // 128         # 8 row tiles
LAG = 6                # cnt-matmul lag (iterations) behind score matmuls
WARM = 5               # extra warm-up repeats of the first score-tile MMs
NDVE = 3               # superchunks whose row-rank runs as DVE dumps (rest ACT)

_cache = {}


def _build_nc():
    import concourse.bacc as bacc
    import concourse.mybir as mybir
    from concourse.tile import TileContext

    f32 = mybir.dt.float32
    f16 = mybir.dt.float16
    bf16 = mybir.dt.bfloat16

    Sign = mybir.ActivationFunctionType.Sign
    ADD = mybir.AluOpType.add
    MAX = mybir.AluOpType.max
    LT = mybir.AluOpType.is_lt

    nc = bacc.Bacc(None)

    imT = nc.declare_dram_parameter("imT", [D, RL], bf16, isOutput=False)
    sT = nc.declare_dram_parameter("sT", [D, N], bf16, isOutput=False)
    diag_r = nc.declare_dram_parameter("diag_r", [128, NT], f32, isOutput=False)
    dcb = nc.declare_dram_parameter("dcb", [128, N], f16, isOutput=False)
    negeye = nc.declare_dram_parameter("negeye", [128, 128], f32, isOutput=False)
    s1_o = nc.declare_dram_parameter("s1", [128, NT * NSC], f32, isOutput=True)
    cnt2_o = nc.declare_dram_parameter("cnt2", [1, N], f32, isOutput=True)
    cmax_o = nc.declare_dram_parameter("cmax", [128, N], f16, isOutput=True)
    rm_o = nc.declare_dram_parameter("rm", [128, NT * SC_W], f16, isOutput=True)
    c0_o = nc.declare_dram_parameter("c0", [128, NDVE * NT * SC_W], bf16,
                                     isOutput=True)

    with TileContext(nc) as tc:
        with (
            tc.tile_pool(name="consts", bufs=1) as cpool,
            tc.tile_pool(name="data", bufs=1) as dpool,
            tc.tile_pool(name="ps", bufs=3, space="PSUM") as pspool,
            tc.tile_pool(name="pcnt", bufs=1, space="PSUM") as pcpool,
            tc.tile_pool(name="s16", bufs=4) as spool,
            tc.tile_pool(name="ind", bufs=LAG + 2) as ipool,
            tc.tile_pool(name="c0t", bufs=3) as c0pool,
            tc.tile_pool(name="trash", bufs=2) as tpool,
            tc.tile_pool(name="outs", bufs=1) as opool,
        ):
            t_negeye = cpool.tile([128, 128], f32, tag="negeye")
            nc.sync.dma_start(out=t_negeye[:], in_=negeye[:])
            t_dr = cpool.tile([128, NT], f32, tag="dr")
            nc.sync.dma_start(out=t_dr[:], in_=diag_r[:])
            t_ones = cpool.tile([128, 1], bf16, tag="ones")
            nc.vector.memset(t_ones[:], 1.0)

            t_imT = []
            for k in range(2):
                t = dpool.tile([128, RL], bf16, tag=f"imT{k}")
                nc.sync.dma_start(out=t[:], in_=imT[k * 128:(k + 1) * 128, :])
                t_imT.append(t)
            t_dcb = dpool.tile([128, N], f16, tag="dcb")
            t_sT = {}
            for k in range(2):
                for b in range(NSC):
                    t = dpool.tile([128, SC_W], bf16, tag=f"sT{k}_{b}")
                    nc.sync.dma_start(
                        out=t[:],
                        in_=sT[k * 128:(k + 1) * 128, b * SC_W:(b + 1) * SC_W],
                    )
                    t_sT[(k, b)] = t
                    if k == 0:
                        nc.sync.dma_start(
                            out=t_dcb[:, b * SC_W:(b + 1) * SC_W],
                            in_=dcb[:, b * SC_W:(b + 1) * SC_W],
                        )

            t_s1 = opool.tile([128, NT * NSC], f32, tag="s1")
            t_cnt2 = opool.tile([1, N], f32, tag="cnt2")
            t_cmax = opool.tile([128, N], f16, tag="cmax")
            t_rm = opool.tile([128, NT * SC_W], f16, tag="rm")
            nc.gpsimd.memset(t_s1[:], 0.0)

            def score_mms(ps, sc, t, reps=1):
                # k-outer: one weight load per (k, t), chunk MMs share it
                for _ in range(reps):
                    for k in range(2):
                        for c in range(SC_W // 512):
                            nc.tensor.matmul(
                                ps[:, c * 512:(c + 1) * 512],
                                lhsT=t_imT[k][:, t * 128:(t + 1) * 128],
                                rhs=t_sT[(k, sc)][:, c * 512:(c + 1) * 512],
                                start=(k == 0),
                                stop=(k == 1),
                            )

            pend = []  # (ind_tile, sc, t) awaiting cnt matmuls
            pcs = {}

            def flush_cnt(limit):
                while len(pend) > limit:
                    ind, psc, pt = pend.pop(0)
                    pc = pcs[psc]
                    for c in range(SC_W // 512):
                        nc.tensor.matmul(
                            pc[0:1, c * 512:(c + 1) * 512],
                            lhsT=t_ones[:],
                            rhs=ind[:, c * 512:(c + 1) * 512],
                            start=(pt == 0),
                            stop=(pt == NT - 1),
                        )
                    if pt == NT - 1:
                        sl = slice(psc * SC_W, (psc + 1) * SC_W)
                        nc.scalar.copy(t_cnt2[0:1, sl], pc[0:1, :])
                        nc.sync.dma_start(out=cnt2_o[0:1, sl], in_=t_cnt2[0:1, sl])

            first = True
            for sc in range(NSC):
                pcs[sc] = pcpool.tile([1, SC_W], f32, tag="pcnt", name=f"pc{sc}")
                for t in range(NT):
                    ps = pspool.tile([128, SC_W], f32, tag="ps")
                    if first:
                        score_mms(ps, sc, t, reps=WARM)
                        first = False
                    score_mms(ps, sc, t)
                    if sc == 0:
                        off = t * 128
                        nc.vector.tensor_tensor(
                            ps[:, off:off + 128], ps[:, off:off + 128],
                            t_negeye[:], ADD,
                        )
                    # single PSUM->SBUF pass: fp16 copy of the score tile
                    s16 = spool.tile([128, SC_W], f16, tag="s16")
                    nc.scalar.copy(s16[:], ps[:])

                    idx = t * NSC + sc
                    if sc < NDVE:
                        # row-rank indicator: 4x tensor_scalar, summed on host
                        c0 = c0pool.tile([128, SC_W], bf16, tag="c0")
                        nc.vector.tensor_scalar(
                            out=c0[:], in0=s16[:],
                            scalar1=t_dr[:, t:t + 1], scalar2=None, op0=LT,
                        )
                        q = sc * NT + t
                        nc.sync.dma_start(
                            out=c0_o[:, q * SC_W:(q + 1) * SC_W], in_=c0[:])
                    else:
                        # row-rank via ACT: accum_out = sum(sign(d_i - S))
                        trash_a = tpool.tile([128, SC_W], bf16, tag="trash_a")
                        nc.scalar.activation(
                            trash_a[:], ps[:], Sign,
                            bias=t_dr[:, t:t + 1], scale=-1.0,
                            accum_out=t_s1[:, idx:idx + 1],
                        )
                    # rowmax accumulate over sc (in place, 2x; first is a copy)
                    rsl = slice(t * SC_W, (t + 1) * SC_W)
                    if sc == 0:
                        nc.vector.tensor_copy(t_rm[:, rsl], s16[:])
                    else:
                        nc.vector.tensor_tensor(
                            t_rm[:, rsl], t_rm[:, rsl], s16[:], MAX)
                    if sc == NSC - 1:
                        nc.sync.dma_start(out=rm_o[:, rsl], in_=t_rm[:, rsl])
                    # col indicator (S < d_j) -> bf16, feeds PE partition-sum
                    ind = ipool.tile([128, SC_W], bf16, tag="ind")
                    nc.vector.tensor_tensor(
                        ind[:], s16[:], t_dcb[:, sc * SC_W:(sc + 1) * SC_W], LT,
                    )
                    # colmax accumulate (in place; first touch is a copy)
                    csl = slice(sc * SC_W, (sc + 1) * SC_W)
                    if t == 0:
                        nc.vector.tensor_copy(t_cmax[:, csl], s16[:])
                    else:
                        nc.vector.tensor_tensor(
                            t_cmax[:, csl], t_cmax[:, csl], s16[:], MAX)
                    if t == NT - 1:
                        nc.sync.dma_start(out=cmax_o[:, csl], in_=t_cmax[:, csl])
                    pend.append((ind, sc, t))
                    flush_cnt(LAG)
            flush_cnt(0)

            nc.sync.dma_start(out=s1_o[:], in_=t_s1[:])

    nc.finalize()
    return nc


def _get_nc():
    if "nc" not in _cache:
        _cache["nc"] = _build_nc()
    return _cache["nc"]


def make_in_maps(im, s):
    imb = np.asarray(im, dtype=np.float32).astype(ml_dtypes.bfloat16)
    sb = np.asarray(s, dtype=np.float32).astype(ml_dtypes.bfloat16)
    imb32 = imb.astype(np.float32)
    sb32 = sb.astype(np.float32)
    diag = np.einsum("ij,ij->i", imb32, sb32).astype(np.float32)
    sT_full = np.ascontiguousarray(sb32.T)
    negeye = np.where(np.eye(128, dtype=bool), NEG, np.float32(0.0)).astype(np.float32)
    in_maps = []
    for r in range(NCORES):
        lo = r * RL
        rolled_diag = np.roll(diag, -lo)
        in_maps.append({
            "imT": np.ascontiguousarray(imb32[lo:lo + RL].T).astype(ml_dtypes.bfloat16),
            "sT": np.roll(sT_full, -lo, axis=1).astype(ml_dtypes.bfloat16),
            "diag_r": np.ascontiguousarray(diag[lo:lo + RL].reshape(NT, 128).T),
            "dcb": np.ascontiguousarray(np.broadcast_to(
                rolled_diag.astype(np.float16)[None, :], (128, N))),
            "negeye": negeye,
        })
    return in_maps, diag


def finish(results, diag):
    """Host-side reduction of the per-core stats to the scalar loss."""
    diag64 = diag.astype(np.float64)
    total = 0.0
    cnt2_sum = np.zeros(N, dtype=np.float64)
    cmax_g = np.full(N, -np.inf, dtype=np.float64)
    for r in range(NCORES):
        lo = r * RL
        s1 = results[r]["s1"].astype(np.float64)      # [128, NT*NSC] sign sums
        cnt2 = results[r]["cnt2"].astype(np.float64)  # [1, N] counts
        cmax = results[r]["cmax"].astype(np.float64)  # [128, N]
        rm = results[r]["rm"].astype(np.float64)      # [128, NT*SC_W]
        c0 = results[r]["c0"].astype(np.float64)      # [128, NDVE*NT*SC_W]
        # slot [p, t*NSC+sc] ; local row i = t*128 + p
        # rank1+1 = sum_{sc<NDVE} indicators + sum_{sc>=NDVE} (SC_W+signsum)/2
        s1m = s1.reshape(128, NT, NSC)[:, :, NDVE:]
        cnt1 = ((SC_W + s1m) / 2.0).sum(axis=2)
        cnt1 += c0.reshape(128, NDVE, NT, SC_W).sum(axis=(1, 3))
        cnt1 = cnt1.T.reshape(RL)
        rmax_row = rm.reshape(128, NT, SC_W).max(axis=2).T.reshape(RL)
        d_loc = diag64[lo:lo + RL]
        total += np.sum(np.maximum(MARGIN + rmax_row - d_loc, 0.0) / cnt1)
        # columns: rotated col j' -> global j = (lo + j') % N
        jj = (lo + np.arange(N)) % N
        cnt2_sum[jj] += cnt2[0]
        cmax_g[jj] = np.maximum(cmax_g[jj], cmax.max(axis=0))
    cnt2_tot = cnt2_sum  # = rank2 + 1 (owning core's mask counts once)
    total += np.sum(np.maximum(MARGIN + cmax_g - diag64, 0.0) / cnt2_tot)
    return np.array(total, dtype=np.float32)


def run_on_hw(im, s, trace=False):
    from concourse.bass_utils import run_bass_kernel_spmd

    in_maps, diag = make_in_maps(im, s)
    nc = _get_nc()
    out = run_bass_kernel_spmd(nc, in_maps, list(range(NCORES)), trace=trace)
    return finish(out.results, diag), out


def kernel(im, s):
    result, _ = run_on_hw(im, s, trace=False)
    return result


# revision 12
# speedup vs baseline: 1.2721x; 1.2721x over previous
"""Trainium2 Bass kernel for the rank-weighted hard-negative hinge loss.

Math (reference):
    scores = im @ s.T                         # [N, N]
    diag   = diagonal(scores)
    rank1[i] = #{j : scores[i,j] < diag[i]}   (row rank of diag)
    rank2[j] = #{i : scores[i,j] < diag[j]}   (col rank of diag)
    cost_s  = 1/(rank1+1) * max_j!=i relu(M + scores[i,j] - diag[i])
    cost_im = 1/(rank2+1) * max_i!=j relu(M + scores[i,j] - diag[j])
    loss = sum(cost_s) + sum(cost_im)

v4 strategy:
  - Scores from bf16-rounded inputs, fp32 PSUM accumulation (bf16 matmuls
    are 4x fp32 on the PE); diag from the same bf16 inputs keeps every
    comparison consistent (sim: rel err ~1.5e-3).
  - ACT owns the PSUM reads: a fp16 Copy conversion per tile, plus
    Sign(+accum) row-rank counts for superchunks 3..7.
  - DVE consumes the fp16 copy with 2x tensor_tensor only: ind2 = (S < d_j)
    is_lt, colmax and rowmax as in-place max accumulators (first touch is a
    4x tensor_copy, which also replaces the big memsets). Row counts for
    superchunks 0..2 are raw 4x tensor_scalar indicators DMA'd out and
    summed on the host (DMA engines are otherwise idle).
  - cnt2 = partition sums of ind2 on the PE (ones-matmul, accumulated over
    row tiles per superchunk). Cnt matmuls are emitted LAG iterations late
    so they never stall the PE on the DVE pipeline; a warm-up burst plus
    deeper PSUM buffering (3 score tiles) keeps the PE HAM at full clock.
  - Outputs are DMA'd in chunks as soon as their accumulation closes, so
    almost nothing is left for the tail.
  - The diagonal is masked by adding -1e30 into PSUM; it deterministically
    counts as "below diag" in both rank counts, yielding rank+1 exactly.

Sharding: core r owns rows [r*1024, (r+1)*1024). Each core receives s.T with
columns rotated left by r*1024 so the diagonal block sits at local column
offset = local row index on every core. Column stats are un-rotated on the
host, which also does the final reduction across cores.
"""

import numpy as np
import ml_dtypes

N = 8192
D = 256
NCORES = 8
RL = N // NCORES  # rows per core
MARGIN = 0.2
NEG = np.float32(-1.0e30)

SC_W = 1024            # column superchunk width
NSC = N // SC_W        # 8 superchunks
NT = RL // 128         # 8 row tiles
LAG = 6                # cnt-matmul lag (iterations) behind score matmuls
WARM = 5               # extra warm-up repeats of the first score-tile MMs
NDVE = 2               # superchunks whose row-rank runs as DVE dumps (rest ACT)

_cache = {}


def _build_nc():
    import concourse.bacc as bacc
    import concourse.mybir as mybir
    from concourse.tile import TileContext

    f32 = mybir.dt.float32
    f16 = mybir.dt.float16
    bf16 = mybir.dt.bfloat16

    Sign = mybir.ActivationFunctionType.Sign
    ADD = mybir.AluOpType.add
    MAX = mybir.AluOpType.max
    LT = mybir.AluOpType.is_lt

    nc = bacc.Bacc(None)

    imT = nc.declare_dram_parameter("imT", [D, RL], bf16, isOutput=False)
    sT = nc.declare_dram_parameter("sT", [D, N], bf16, isOutput=False)
    diag_r = nc.declare_dram_parameter("diag_r", [128, NT], f32, isOutput=False)
    dcb = nc.declare_dram_parameter("dcb", [128, N], f16, isOutput=False)
    negeye = nc.declare_dram_parameter("negeye", [128, 128], f32, isOutput=False)
    s1_o = nc.declare_dram_parameter("s1", [128, NT * NSC], f32, isOutput=True)
    cnt2_o = nc.declare_dram_parameter("cnt2", [1, N], f32, isOutput=True)
    cmax_o = nc.declare_dram_parameter("cmax", [128, N], f16, isOutput=True)
    rm_o = nc.declare_dram_parameter("rm", [128, NT * SC_W], f16, isOutput=True)
    c0_o = nc.declare_dram_parameter("c0", [128, NDVE * NT * SC_W], bf16,
                                     isOutput=True)

    with TileContext(nc) as tc:
        with (
            tc.tile_pool(name="consts", bufs=1) as cpool,
            tc.tile_pool(name="data", bufs=1) as dpool,
            tc.tile_pool(name="ps", bufs=3, space="PSUM") as pspool,
            tc.tile_pool(name="pcnt", bufs=1, space="PSUM") as pcpool,
            tc.tile_pool(name="s16", bufs=4) as spool,
            tc.tile_pool(name="ind", bufs=LAG + 2) as ipool,
            tc.tile_pool(name="c0t", bufs=3) as c0pool,
            tc.tile_pool(name="trash", bufs=2) as tpool,
            tc.tile_pool(name="outs", bufs=1) as opool,
        ):
            t_negeye = cpool.tile([128, 128], f32, tag="negeye")
            nc.sync.dma_start(out=t_negeye[:], in_=negeye[:])
            t_dr = cpool.tile([128, NT], f32, tag="dr")
            nc.sync.dma_start(out=t_dr[:], in_=diag_r[:])
            t_ones = cpool.tile([128, 1], bf16, tag="ones")
            nc.vector.memset(t_ones[:], 1.0)

            t_imT = []
            for k in range(2):
                t = dpool.tile([128, RL], bf16, tag=f"imT{k}")
                nc.sync.dma_start(out=t[:], in_=imT[k * 128:(k + 1) * 128, :])
                t_imT.append(t)
            t_dcb = dpool.tile([128, N], f16, tag="dcb")
            t_sT = {}
            # b-outer so superchunk 0's operands land first and compute
            # starts while the rest of the inputs stream in
            for b in range(NSC):
                for k in range(2):
                    t = dpool.tile([128, SC_W], bf16, tag=f"sT{k}_{b}")
                    nc.sync.dma_start(
                        out=t[:],
                        in_=sT[k * 128:(k + 1) * 128, b * SC_W:(b + 1) * SC_W],
                    )
                    t_sT[(k, b)] = t
                nc.sync.dma_start(
                    out=t_dcb[:, b * SC_W:(b + 1) * SC_W],
                    in_=dcb[:, b * SC_W:(b + 1) * SC_W],
                )

            t_s1 = opool.tile([128, NT * NSC], f32, tag="s1")
            t_cnt2 = opool.tile([1, N], f32, tag="cnt2")
            t_cmax = opool.tile([128, N], f16, tag="cmax")
            t_rm = opool.tile([128, NT * SC_W], f16, tag="rm")
            nc.gpsimd.memset(t_s1[:], 0.0)

            def score_mms(ps, sc, t, reps=1):
                # k-outer: one weight load per (k, t), chunk MMs share it
                for _ in range(reps):
                    for k in range(2):
                        for c in range(SC_W // 512):
                            nc.tensor.matmul(
                                ps[:, c * 512:(c + 1) * 512],
                                lhsT=t_imT[k][:, t * 128:(t + 1) * 128],
                                rhs=t_sT[(k, sc)][:, c * 512:(c + 1) * 512],
                                start=(k == 0),
                                stop=(k == 1),
                            )

            pend = []  # (ind_tile, sc, t) awaiting cnt matmuls
            pcs = {}

            def flush_cnt(limit):
                while len(pend) > limit:
                    ind, psc, pt = pend.pop(0)
                    pc = pcs[psc]
                    for c in range(SC_W // 512):
                        nc.tensor.matmul(
                            pc[0:1, c * 512:(c + 1) * 512],
                            lhsT=t_ones[:],
                            rhs=ind[:, c * 512:(c + 1) * 512],
                            start=(pt == 0),
                            stop=(pt == NT - 1),
                        )
                    if pt == NT - 1:
                        sl = slice(psc * SC_W, (psc + 1) * SC_W)
                        nc.scalar.copy(t_cnt2[0:1, sl], pc[0:1, :])
                        nc.sync.dma_start(out=cnt2_o[0:1, sl], in_=t_cnt2[0:1, sl])

            first = True
            for sc in range(NSC):
                pcs[sc] = pcpool.tile([1, SC_W], f32, tag="pcnt", name=f"pc{sc}")
                for t in range(NT):
                    ps = pspool.tile([128, SC_W], f32, tag="ps")
                    if first:
                        score_mms(ps, sc, t, reps=WARM)
                        first = False
                    score_mms(ps, sc, t)
                    if sc == 0:
                        off = t * 128
                        nc.vector.tensor_tensor(
                            ps[:, off:off + 128], ps[:, off:off + 128],
                            t_negeye[:], ADD,
                        )
                    # single PSUM->SBUF pass: fp16 copy of the score tile
                    s16 = spool.tile([128, SC_W], f16, tag="s16")
                    nc.scalar.copy(s16[:], ps[:])

                    idx = t * NSC + sc
                    if sc < NDVE:
                        # row-rank indicator: 4x tensor_scalar, summed on host
                        c0 = c0pool.tile([128, SC_W], bf16, tag="c0")
                        nc.vector.tensor_scalar(
                            out=c0[:], in0=s16[:],
                            scalar1=t_dr[:, t:t + 1], scalar2=None, op0=LT,
                        )
                        q = sc * NT + t
                        nc.sync.dma_start(
                            out=c0_o[:, q * SC_W:(q + 1) * SC_W], in_=c0[:])
                    else:
                        # row-rank via ACT: accum_out = sum(sign(d_i - S))
                        trash_a = tpool.tile([128, SC_W], bf16, tag="trash_a")
                        nc.scalar.activation(
                            trash_a[:], ps[:], Sign,
                            bias=t_dr[:, t:t + 1], scale=-1.0,
                            accum_out=t_s1[:, idx:idx + 1],
                        )
                    # rowmax accumulate over sc (in place, 2x; first is a copy)
                    rsl = slice(t * SC_W, (t + 1) * SC_W)
                    if sc == 0:
                        nc.vector.tensor_copy(t_rm[:, rsl], s16[:])
                    else:
                        nc.vector.tensor_tensor(
                            t_rm[:, rsl], t_rm[:, rsl], s16[:], MAX)
                    if sc == NSC - 1:
                        nc.sync.dma_start(out=rm_o[:, rsl], in_=t_rm[:, rsl])
                    # col indicator (S < d_j) -> bf16, feeds PE partition-sum
                    ind = ipool.tile([128, SC_W], bf16, tag="ind")
                    nc.vector.tensor_tensor(
                        ind[:], s16[:], t_dcb[:, sc * SC_W:(sc + 1) * SC_W], LT,
                    )
                    # colmax accumulate (in place; first touch is a copy)
                    csl = slice(sc * SC_W, (sc + 1) * SC_W)
                    if t == 0:
                        nc.vector.tensor_copy(t_cmax[:, csl], s16[:])
                    else:
                        nc.vector.tensor_tensor(
                            t_cmax[:, csl], t_cmax[:, csl], s16[:], MAX)
                    if t == NT - 1:
                        nc.sync.dma_start(out=cmax_o[:, csl], in_=t_cmax[:, csl])
                    pend.append((ind, sc, t))
                    flush_cnt(LAG)
            flush_cnt(0)

            nc.sync.dma_start(out=s1_o[:], in_=t_s1[:])

    nc.finalize()
    return nc


def _get_nc():
    if "nc" not in _cache:
        _cache["nc"] = _build_nc()
    return _cache["nc"]


def make_in_maps(im, s):
    imb = np.asarray(im, dtype=np.float32).astype(ml_dtypes.bfloat16)
    sb = np.asarray(s, dtype=np.float32).astype(ml_dtypes.bfloat16)
    imb32 = imb.astype(np.float32)
    sb32 = sb.astype(np.float32)
    diag = np.einsum("ij,ij->i", imb32, sb32).astype(np.float32)
    sT_full = np.ascontiguousarray(sb32.T)
    negeye = np.where(np.eye(128, dtype=bool), NEG, np.float32(0.0)).astype(np.float32)
    in_maps = []
    for r in range(NCORES):
        lo = r * RL
        rolled_diag = np.roll(diag, -lo)
        in_maps.append({
            "imT": np.ascontiguousarray(imb32[lo:lo + RL].T).astype(ml_dtypes.bfloat16),
            "sT": np.roll(sT_full, -lo, axis=1).astype(ml_dtypes.bfloat16),
            "diag_r": np.ascontiguousarray(diag[lo:lo + RL].reshape(NT, 128).T),
            "dcb": np.ascontiguousarray(np.broadcast_to(
                rolled_diag.astype(np.float16)[None, :], (128, N))),
            "negeye": negeye,
        })
    return in_maps, diag


def finish(results, diag):
    """Host-side reduction of the per-core stats to the scalar loss."""
    diag64 = diag.astype(np.float64)
    total = 0.0
    cnt2_sum = np.zeros(N, dtype=np.float64)
    cmax_g = np.full(N, -np.inf, dtype=np.float64)
    for r in range(NCORES):
        lo = r * RL
        s1 = results[r]["s1"].astype(np.float64)      # [128, NT*NSC] sign sums
        cnt2 = results[r]["cnt2"].astype(np.float64)  # [1, N] counts
        cmax = results[r]["cmax"].astype(np.float64)  # [128, N]
        rm = results[r]["rm"].astype(np.float64)      # [128, NT*SC_W]
        c0 = results[r]["c0"].astype(np.float64)      # [128, NDVE*NT*SC_W]
        # slot [p, t*NSC+sc] ; local row i = t*128 + p
        # rank1+1 = sum_{sc<NDVE} indicators + sum_{sc>=NDVE} (SC_W+signsum)/2
        s1m = s1.reshape(128, NT, NSC)[:, :, NDVE:]
        cnt1 = ((SC_W + s1m) / 2.0).sum(axis=2)
        cnt1 += c0.reshape(128, NDVE, NT, SC_W).sum(axis=(1, 3))
        cnt1 = cnt1.T.reshape(RL)
        rmax_row = rm.reshape(128, NT, SC_W).max(axis=2).T.reshape(RL)
        d_loc = diag64[lo:lo + RL]
        total += np.sum(np.maximum(MARGIN + rmax_row - d_loc, 0.0) / cnt1)
        # columns: rotated col j' -> global j = (lo + j') % N
        jj = (lo + np.arange(N)) % N
        cnt2_sum[jj] += cnt2[0]
        cmax_g[jj] = np.maximum(cmax_g[jj], cmax.max(axis=0))
    cnt2_tot = cnt2_sum  # = rank2 + 1 (owning core's mask counts once)
    total += np.sum(np.maximum(MARGIN + cmax_g - diag64, 0.0) / cnt2_tot)
    return np.array(total, dtype=np.float32)


def run_on_hw(im, s, trace=False):
    from concourse.bass_utils import run_bass_kernel_spmd

    in_maps, diag = make_in_maps(im, s)
    nc = _get_nc()
    out = run_bass_kernel_spmd(nc, in_maps, list(range(NCORES)), trace=trace)
    return finish(out.results, diag), out


def kernel(im, s):
    result, _ = run_on_hw(im, s, trace=False)
    return result


# revision 18
# speedup vs baseline: 1.3535x; 1.0640x over previous
"""Trainium2 Bass kernel for the rank-weighted hard-negative hinge loss.

Math (reference):
    scores = im @ s.T                         # [N, N]
    diag   = diagonal(scores)
    rank1[i] = #{j : scores[i,j] < diag[i]}   (row rank of diag)
    rank2[j] = #{i : scores[i,j] < diag[j]}   (col rank of diag)
    cost_s  = 1/(rank1+1) * max_j!=i relu(M + scores[i,j] - diag[i])
    cost_im = 1/(rank2+1) * max_i!=j relu(M + scores[i,j] - diag[j])
    loss = sum(cost_s) + sum(cost_im)

v4 strategy:
  - Scores from bf16-rounded inputs, fp32 PSUM accumulation (bf16 matmuls
    are 4x fp32 on the PE); diag from the same bf16 inputs keeps every
    comparison consistent (sim: rel err ~1.5e-3).
  - ACT owns the PSUM reads: a fp16 Copy conversion per tile, plus
    Sign(+accum) row-rank counts for superchunks 3..7.
  - DVE consumes the fp16 copy with 2x tensor_tensor only: ind2 = (S < d_j)
    is_lt, colmax and rowmax as in-place max accumulators (first touch is a
    4x tensor_copy, which also replaces the big memsets). Row counts for
    superchunks 0..2 are raw 4x tensor_scalar indicators DMA'd out and
    summed on the host (DMA engines are otherwise idle).
  - cnt2 = partition sums of ind2 on the PE (ones-matmul, accumulated over
    row tiles per superchunk). Cnt matmuls are emitted LAG iterations late
    so they never stall the PE on the DVE pipeline; a warm-up burst plus
    deeper PSUM buffering (3 score tiles) keeps the PE HAM at full clock.
  - Outputs are DMA'd in chunks as soon as their accumulation closes, so
    almost nothing is left for the tail.
  - The diagonal is masked by adding -1e30 into PSUM; it deterministically
    counts as "below diag" in both rank counts, yielding rank+1 exactly.

Sharding: core r owns rows [r*1024, (r+1)*1024). Each core receives s.T with
columns rotated left by r*1024 so the diagonal block sits at local column
offset = local row index on every core. Column stats are un-rotated on the
host, which also does the final reduction across cores.
"""

import numpy as np
import ml_dtypes

N = 8192
D = 256
NCORES = 8
RL = N // NCORES  # rows per core
MARGIN = 0.2
NEG = np.float32(-1.0e30)

SC_W = 1024            # column superchunk width
NSC = N // SC_W        # 8 superchunks
NT = RL // 128         # 8 row tiles
LAG = 3                # cnt-matmul lag (iterations) behind score matmuls
WARM = 5               # warm-up repeats of scratch MMs (no DMA dependency)
DUMP_SCS = (0, 3, 6)   # superchunks whose row-rank runs as DVE dumps (rest ACT)

_cache = {}


def _build_nc():
    import concourse.bacc as bacc
    import concourse.mybir as mybir
    from concourse.tile import TileContext

    f32 = mybir.dt.float32
    f16 = mybir.dt.float16
    bf16 = mybir.dt.bfloat16

    Sign = mybir.ActivationFunctionType.Sign
    ADD = mybir.AluOpType.add
    MAX = mybir.AluOpType.max
    LT = mybir.AluOpType.is_lt

    nc = bacc.Bacc(None)

    imT = nc.declare_dram_parameter("imT", [D, RL], bf16, isOutput=False)
    sT = nc.declare_dram_parameter("sT", [D, N], bf16, isOutput=False)
    diag_r = nc.declare_dram_parameter("diag_r", [128, NT], f32, isOutput=False)
    dcb = nc.declare_dram_parameter("dcb", [128, N], f16, isOutput=False)
    negeye = nc.declare_dram_parameter("negeye", [128, 128], f32, isOutput=False)
    s1_o = nc.declare_dram_parameter("s1", [128, NT * NSC], f32, isOutput=True)
    cnt2_o = nc.declare_dram_parameter("cnt2", [1, N], f32, isOutput=True)
    cmax_o = nc.declare_dram_parameter("cmax", [128, N], f16, isOutput=True)
    rm_o = nc.declare_dram_parameter("rm", [128, NT * SC_W], f16, isOutput=True)
    c0_o = nc.declare_dram_parameter("c0", [128, len(DUMP_SCS) * NT * SC_W], bf16,
                                     isOutput=True)

    with TileContext(nc) as tc:
        with (
            tc.tile_pool(name="consts", bufs=1) as cpool,
            tc.tile_pool(name="data", bufs=1) as dpool,
            tc.tile_pool(name="ps", bufs=3, space="PSUM") as pspool,
            tc.tile_pool(name="pcnt", bufs=1, space="PSUM") as pcpool,
            tc.tile_pool(name="s16", bufs=4) as spool,
            tc.tile_pool(name="ind", bufs=LAG + 2) as ipool,
            tc.tile_pool(name="c0t", bufs=3) as c0pool,
            tc.tile_pool(name="trash", bufs=2) as tpool,
            tc.tile_pool(name="outs", bufs=1) as opool,
        ):
            t_negeye = cpool.tile([128, 128], f32, tag="negeye")
            nc.sync.dma_start(out=t_negeye[:], in_=negeye[:])
            t_dr = cpool.tile([128, NT], f32, tag="dr")
            nc.sync.dma_start(out=t_dr[:], in_=diag_r[:])
            t_ones = cpool.tile([128, 1], bf16, tag="ones")
            nc.vector.memset(t_ones[:], 1.0)

            t_imT = []
            for k in range(2):
                t = dpool.tile([128, RL], bf16, tag=f"imT{k}")
                nc.sync.dma_start(out=t[:], in_=imT[k * 128:(k + 1) * 128, :])
                t_imT.append(t)
            t_dcb = dpool.tile([128, N], f16, tag="dcb")
            t_sT = {}
            # b-outer so superchunk 0's operands land first and compute
            # starts while the rest of the inputs stream in
            for b in range(NSC):
                for k in range(2):
                    t = dpool.tile([128, SC_W], bf16, tag=f"sT{k}_{b}")
                    nc.sync.dma_start(
                        out=t[:],
                        in_=sT[k * 128:(k + 1) * 128, b * SC_W:(b + 1) * SC_W],
                    )
                    t_sT[(k, b)] = t
                nc.sync.dma_start(
                    out=t_dcb[:, b * SC_W:(b + 1) * SC_W],
                    in_=dcb[:, b * SC_W:(b + 1) * SC_W],
                )

            t_s1 = opool.tile([128, NT * NSC], f32, tag="s1")
            t_cnt2 = opool.tile([1, N], f32, tag="cnt2")
            t_cmax = opool.tile([128, N], f16, tag="cmax")
            t_rm = opool.tile([128, NT * SC_W], f16, tag="rm")
            nc.gpsimd.memset(t_s1[:], 0.0)

            def score_mms(ps, sc, t, reps=1):
                # k-outer: one weight load per (k, t), chunk MMs share it
                for _ in range(reps):
                    for k in range(2):
                        for c in range(SC_W // 512):
                            nc.tensor.matmul(
                                ps[:, c * 512:(c + 1) * 512],
                                lhsT=t_imT[k][:, t * 128:(t + 1) * 128],
                                rhs=t_sT[(k, sc)][:, c * 512:(c + 1) * 512],
                                start=(k == 0),
                                stop=(k == 1),
                            )

            pend = []  # (ind_tile, sc, t) awaiting cnt matmuls
            pcs = {}

            def flush_cnt(limit):
                while len(pend) > limit:
                    ind, psc, pt = pend.pop(0)
                    pc = pcs[psc]
                    for c in range(SC_W // 512):
                        nc.tensor.matmul(
                            pc[0:1, c * 512:(c + 1) * 512],
                            lhsT=t_ones[:],
                            rhs=ind[:, c * 512:(c + 1) * 512],
                            start=(pt == 0),
                            stop=(pt == NT - 1),
                        )
                    if pt == NT - 1:
                        sl = slice(psc * SC_W, (psc + 1) * SC_W)
                        nc.scalar.copy(t_cnt2[0:1, sl], pc[0:1, :])
                        nc.sync.dma_start(out=cnt2_o[0:1, sl], in_=t_cnt2[0:1, sl])

            # warm-up burst on scratch data (no DMA dependency -> runs during
            # the input DMA head and flips the PE HAM to full clock)
            t_wsc = cpool.tile([128, 512], bf16, tag="wsc")
            nc.vector.memset(t_wsc[:], 0.5)
            ps_w = pspool.tile([128, SC_W], f32, tag="ps", name="ps_warm")
            for _ in range(WARM * 4):
                nc.tensor.matmul(ps_w[:, 0:512], lhsT=t_wsc[:, 0:128],
                                 rhs=t_wsc[:], start=True, stop=True)

            for sc in range(NSC):
                pcs[sc] = pcpool.tile([1, SC_W], f32, tag="pcnt", name=f"pc{sc}")
                for t in range(NT):
                    ps = pspool.tile([128, SC_W], f32, tag="ps")
                    score_mms(ps, sc, t)
                    if sc == 0:
                        off = t * 128
                        nc.vector.tensor_tensor(
                            ps[:, off:off + 128], ps[:, off:off + 128],
                            t_negeye[:], ADD,
                        )
                    # single PSUM->SBUF pass: fp16 copy of the score tile
                    s16 = spool.tile([128, SC_W], f16, tag="s16")
                    nc.scalar.copy(s16[:], ps[:])

                    idx = t * NSC + sc
                    if sc in DUMP_SCS:
                        # row-rank indicator: 4x tensor_scalar, summed on host
                        c0 = c0pool.tile([128, SC_W], bf16, tag="c0")
                        nc.vector.tensor_scalar(
                            out=c0[:], in0=s16[:],
                            scalar1=t_dr[:, t:t + 1], scalar2=None, op0=LT,
                        )
                        q = DUMP_SCS.index(sc) * NT + t
                        nc.sync.dma_start(
                            out=c0_o[:, q * SC_W:(q + 1) * SC_W], in_=c0[:])
                    else:
                        # row-rank via ACT: accum_out = sum(sign(d_i - S))
                        trash_a = tpool.tile([128, SC_W], bf16, tag="trash_a")
                        nc.scalar.activation(
                            trash_a[:], ps[:], Sign,
                            bias=t_dr[:, t:t + 1], scale=-1.0,
                            accum_out=t_s1[:, idx:idx + 1],
                        )
                    # rowmax accumulate over sc (in place, 2x; first is a copy)
                    rsl = slice(t * SC_W, (t + 1) * SC_W)
                    if sc == 0:
                        nc.vector.tensor_copy(t_rm[:, rsl], s16[:])
                    else:
                        nc.vector.tensor_tensor(
                            t_rm[:, rsl], t_rm[:, rsl], s16[:], MAX)
                    if sc == NSC - 1:
                        nc.sync.dma_start(out=rm_o[:, rsl], in_=t_rm[:, rsl])
                    # col indicator (S < d_j) -> bf16, feeds PE partition-sum
                    ind = ipool.tile([128, SC_W], bf16, tag="ind")
                    nc.vector.tensor_tensor(
                        ind[:], s16[:], t_dcb[:, sc * SC_W:(sc + 1) * SC_W], LT,
                    )
                    # colmax accumulate (in place; first touch is a copy)
                    csl = slice(sc * SC_W, (sc + 1) * SC_W)
                    if t == 0:
                        nc.vector.tensor_copy(t_cmax[:, csl], s16[:])
                    else:
                        nc.vector.tensor_tensor(
                            t_cmax[:, csl], t_cmax[:, csl], s16[:], MAX)
                    if t == NT - 1:
                        nc.sync.dma_start(out=cmax_o[:, csl], in_=t_cmax[:, csl])
                    pend.append((ind, sc, t))
                    flush_cnt(LAG)
            flush_cnt(0)

            nc.sync.dma_start(out=s1_o[:], in_=t_s1[:])

    nc.finalize()
    return nc


def _get_nc():
    if "nc" not in _cache:
        _cache["nc"] = _build_nc()
    return _cache["nc"]


def make_in_maps(im, s):
    imb = np.asarray(im, dtype=np.float32).astype(ml_dtypes.bfloat16)
    sb = np.asarray(s, dtype=np.float32).astype(ml_dtypes.bfloat16)
    imb32 = imb.astype(np.float32)
    sb32 = sb.astype(np.float32)
    diag = np.einsum("ij,ij->i", imb32, sb32).astype(np.float32)
    sT_full = np.ascontiguousarray(sb32.T)
    negeye = np.where(np.eye(128, dtype=bool), NEG, np.float32(0.0)).astype(np.float32)
    in_maps = []
    for r in range(NCORES):
        lo = r * RL
        rolled_diag = np.roll(diag, -lo)
        in_maps.append({
            "imT": np.ascontiguousarray(imb32[lo:lo + RL].T).astype(ml_dtypes.bfloat16),
            "sT": np.roll(sT_full, -lo, axis=1).astype(ml_dtypes.bfloat16),
            "diag_r": np.ascontiguousarray(diag[lo:lo + RL].reshape(NT, 128).T),
            "dcb": np.ascontiguousarray(np.broadcast_to(
                rolled_diag.astype(np.float16)[None, :], (128, N))),
            "negeye": negeye,
        })
    return in_maps, diag


def finish(results, diag):
    """Host-side reduction of the per-core stats to the scalar loss."""
    diag64 = diag.astype(np.float64)
    total = 0.0
    cnt2_sum = np.zeros(N, dtype=np.float64)
    cmax_g = np.full(N, -np.inf, dtype=np.float64)
    for r in range(NCORES):
        lo = r * RL
        s1 = results[r]["s1"].astype(np.float64)      # [128, NT*NSC] sign sums
        cnt2 = results[r]["cnt2"].astype(np.float64)  # [1, N] counts
        cmax = results[r]["cmax"].astype(np.float64)  # [128, N]
        rm = results[r]["rm"].astype(np.float64)      # [128, NT*SC_W]
        c0 = results[r]["c0"].astype(np.float64)      # [128, ndump*NT*SC_W]
        # slot [p, t*NSC+sc] ; local row i = t*128 + p
        # rank1+1 = sum_{dump scs} indicators + sum_{sign scs} (SC_W+signsum)/2
        sign_scs = [sc for sc in range(NSC) if sc not in DUMP_SCS]
        s1m = s1.reshape(128, NT, NSC)[:, :, sign_scs]
        cnt1 = ((SC_W + s1m) / 2.0).sum(axis=2)
        cnt1 += c0.reshape(128, len(DUMP_SCS), NT, SC_W).sum(axis=(1, 3))
        cnt1 = cnt1.T.reshape(RL)
        rmax_row = rm.reshape(128, NT, SC_W).max(axis=2).T.reshape(RL)
        d_loc = diag64[lo:lo + RL]
        total += np.sum(np.maximum(MARGIN + rmax_row - d_loc, 0.0) / cnt1)
        # columns: rotated col j' -> global j = (lo + j') % N
        jj = (lo + np.arange(N)) % N
        cnt2_sum[jj] += cnt2[0]
        cmax_g[jj] = np.maximum(cmax_g[jj], cmax.max(axis=0))
    cnt2_tot = cnt2_sum  # = rank2 + 1 (owning core's mask counts once)
    total += np.sum(np.maximum(MARGIN + cmax_g - diag64, 0.0) / cnt2_tot)
    return np.array(total, dtype=np.float32)


def run_on_hw(im, s, trace=False):
    from concourse.bass_utils import run_bass_kernel_spmd

    in_maps, diag = make_in_maps(im, s)
    nc = _get_nc()
    out = run_bass_kernel_spmd(nc, in_maps, list(range(NCORES)), trace=trace)
    return finish(out.results, diag), out


def kernel(im, s):
    result, _ = run_on_hw(im, s, trace=False)
    return result
